# revision 7
# baseline (speedup 1.0000x reference)
"""AO layer kernel for Trainium2 (8 NeuronCores, data-parallel over walkers).

Math: out[b,n,a] = ang(a, r) * rad(a, r),  r = pos[b,n] - centers[a]
  rad = sum_p coeffs[a,p] * exp(-exps[a,p] * |r|^2)
  ang = prod_c r_c^powers[a,c],  powers in {0,1,2}

Design (per core, i = flattened (b,n) index, I=2048, 4 i-tiles of 512):
  z[(a,p), i] = W^T RR    (PE bf16 hi/lo 20-row basis, 12 r-tiles/i-tile)
  E = |c|exp(z)           split between ACT (exact exp, bf16 out) and DVE
                          (Schraudolph: weights pre-scaled so z lands in
                          bf16-bit space; clamp+int16-truncate via
                          tensor_scalar_max, bits reinterpreted as bf16)
  rad[a, i]   = S^T E     (PE bf16 accumulate, coeff signs inside S)
  ang[a, i]   = Q^T M     (PE single bf16 hi/lo matmul over the 27-monomial
                          tensor-product basis x^i y^j z^k, 81 rows)
  out = ang*rad           (DVE tensor_mul, ang copied PSUM->SBUF first since
                          TensorTensor may read only one PSUM input)
  Exp ops span 3 r-tiles ([128,1536]) to amortize per-op overhead.
  PSUM: 2x zp[128,1536] (6 banks) + rad[128,512] + ang[128,512].
  Output DRAM layout [128, 2, I] so each i-tile ships as one DMA.
"""

import numpy as np
import ml_dtypes

B, NEL, A, P = 512, 32, 256, 6
NCORES = 8
BS = B // NCORES          # 64 walkers per core
I = BS * NEL              # 2048 i per core
ITILE = 512
NIT = I // ITILE          # 4
NRT = 12                  # r-tiles of 128 (a,p) rows
KZ = 20                   # z basis rows (bf16 hi/lo)
KM = 81                   # monomial basis rows (bf16 hi/lo x3)
LOG2E = 1.4426950408889634
SDELTA = 0.058            # Schraudolph centering

# exp-group -> engine: (it, h, g) in set => DVE Schraudolph, else ACT exp.
DVE_UNITS = {(0, 1, 1), (1, 0, 1), (2, 1, 0), (3, 0, 0)}
# ang PSUM->SBUF copy engine per (it, h): in set => ACT Copy, else DVE copy
ACT_COPY_UNITS = {(0, 0), (2, 1)}

# packed input layouts
ZIN_W = 0                 # [0:1536)       exact-exp weights
ZIN_RR = A * P            # [1536:3584)    rr basis
ZIN_WS = A * P + I        # [3584:5120)    Schraudolph weights
ZIN_COLS = I + 2 * A * P
AIN_Q = 0                 # [0:256)        q81
AIN_M = A                 # [256:2304)     m81
AIN_COLS = A + I

_CACHE = {}


def _bf(v):
    return np.asarray(v, np.float64).astype(ml_dtypes.bfloat16)


def _split_hl(w):
    wh = _bf(w).astype(np.float64)
    wl = _bf(np.asarray(w, np.float64) - wh).astype(np.float64)
    return wh, wl


def _build_nc():
    import concourse.bass as bass
    import concourse.bacc as bacc
    import concourse.tile as tile
    import concourse.mybir as mybir

    f32 = mybir.dt.float32
    bf16 = mybir.dt.bfloat16
    i16 = mybir.dt.int16
    EXP = mybir.ActivationFunctionType.Exp
    CPY = mybir.ActivationFunctionType.Copy
    PSUM = bass.MemorySpace.PSUM

    nc = bacc.Bacc("TRN2", target_bir_lowering=False, debug=False,
                   num_devices=NCORES)

    zin_d = nc.declare_dram_parameter("zin", [KZ, ZIN_COLS], bf16,
                                      isOutput=False)
    ain_d = nc.declare_dram_parameter("ain", [KM, AIN_COLS], bf16,
                                      isOutput=False)
    s_d = nc.declare_dram_parameter("s", [128, NRT * 128], bf16,
                                    isOutput=False)
    out_d = nc.declare_dram_parameter("out", [128, 2, I], bf16, isOutput=True)

    with tile.TileContext(nc) as tc:
        with (
            tc.tile_pool(name="const", bufs=1) as const,
            tc.tile_pool(name="zp", bufs=2, space=PSUM) as zp,
            tc.tile_pool(name="radp", bufs=1, space=PSUM) as radp,
            tc.tile_pool(name="angp", bufs=1, space=PSUM) as angp,
            tc.tile_pool(name="ep", bufs=4) as ep,
            tc.tile_pool(name="ap", bufs=3) as ap,
            tc.tile_pool(name="op", bufs=3) as op,
        ):
            zin_sb = const.tile([KZ, ZIN_COLS], bf16, tag="zin")
            ain_sb = const.tile([KM, AIN_COLS], bf16, tag="ain")
            s_sb = const.tile([128, NRT * 128], bf16, tag="s")
            # ordered for just-in-time arrival (HWDGE is serialized)
            c1 = ZIN_RR + ITILE
            nc.sync.dma_start(zin_sb[:, 0:c1], zin_d[:, 0:c1])
            nc.scalar.dma_start(ain_sb[:, 0:AIN_M + ITILE],
                                ain_d[:, 0:AIN_M + ITILE])
            nc.sync.dma_start(zin_sb[:, c1:], zin_d[:, c1:])
            nc.sync.dma_start(s_sb[:], s_d[:, :])
            nc.scalar.dma_start(ain_sb[:, AIN_M + ITILE:],
                                ain_d[:, AIN_M + ITILE:])

            for it in range(NIT):
                i0 = it * ITILE
                ri = zin_sb[:, ZIN_RR + i0:ZIN_RR + i0 + ITILE]
                for h in range(2):
                    last = it == NIT - 1 and h == 1
                    rad_t = radp.tile([128, ITILE], f32, tag="rad")
                    ang_t = angp.tile([128, ITILE], f32, tag="ang")
                    es = []
                    for g in range(2):
                        dve = (it, h, g) in DVE_UNITS
                        z_t = zp.tile([128, 3 * ITILE], f32, tag="z")
                        for j in range(3):
                            rt = h * 6 + g * 3 + j
                            w0 = (ZIN_WS if dve else ZIN_W) + rt * 128
                            nc.tensor.matmul(
                                z_t[:, j * ITILE:(j + 1) * ITILE],
                                zin_sb[:, w0:w0 + 128], ri,
                                start=True, stop=True)
                        if dve:
                            e_t = ep.tile([128, 3 * ITILE], i16, tag="e")
                            with tc.high_priority(offset=-500):
                                for j in range(3):
                                    sl = slice(j * ITILE, (j + 1) * ITILE)
                                    nc.vector.tensor_scalar_max(
                                        e_t[:, sl], z_t[:, sl], 0.0)
                            es.append(e_t[:].bitcast(bf16))
                        else:
                            e_t = ep.tile([128, 3 * ITILE], bf16, tag="e")
                            nc.scalar.activation(e_t[:], z_t[:], EXP)
                            es.append(e_t[:])
                    # angular: one bf16 hi/lo matmul; deferred priority so it
                    # never delays ready z/rad passes
                    with tc.high_priority(offset=-1000):
                        nc.tensor.matmul(
                            ang_t[:],
                            ain_sb[:, AIN_Q + h * 128:AIN_Q + (h + 1) * 128],
                            ain_sb[:, AIN_M + i0:AIN_M + i0 + ITILE],
                            start=True, stop=True)
                    ang_sb = ap.tile([128, ITILE], bf16, tag="angsb")
                    with tc.high_priority():
                        if (it, h) in ACT_COPY_UNITS:
                            nc.scalar.activation(ang_sb[:], ang_t[:], CPY)
                        else:
                            nc.vector.tensor_copy(ang_sb[:], ang_t[:])
                    # radial: 6 accumulating sign-matmul passes
                    for g in range(2):
                        for j in range(3):
                            rt = h * 6 + g * 3 + j
                            nc.tensor.matmul(
                                rad_t[:],
                                s_sb[:, rt * 128:(rt + 1) * 128],
                                es[g][:, j * ITILE:(j + 1) * ITILE],
                                start=(g == 0 and j == 0),
                                stop=(g == 1 and j == 2))
                    if h == 0:
                        o_t = op.tile([128, 2, ITILE], bf16, tag="o")
                    if last:
                        # tail: ship h0 + most of h1 early; tiny final piece
                        # minimizes the post-compute DMA latency
                        H = 384
                        with tc.high_priority():
                            nc.vector.tensor_mul(
                                o_t[:, 1, 0:H], ang_sb[:, 0:H], rad_t[:, 0:H])
                        nc.scalar.dma_start(
                            out_d[:, 0:1, i0:i0 + ITILE], o_t[:, 0:1, :])
                        nc.scalar.dma_start(
                            out_d[:, 1, i0:i0 + H], o_t[:, 1, 0:H])
                        with tc.high_priority():
                            nc.vector.tensor_mul(
                                o_t[:, 1, H:], ang_sb[:, H:], rad_t[:, H:])
                        nc.sync.dma_start(
                            out_d[:, 1, i0 + H:i0 + ITILE], o_t[:, 1, H:])
                    else:
                        with tc.high_priority():
                            nc.vector.tensor_mul(o_t[:, h, :], ang_sb[:],
                                                 rad_t[:])
                        if h == 1:
                            nc.sync.dma_start(
                                out_d[:, :, i0:i0 + ITILE], o_t[:])

    nc.compile()
    return nc


def _consts(centers, exps, coeffs, powers):
    """Device weight tensors (shared across cores)."""
    al = exps.astype(np.float64)
    co = coeffs.astype(np.float64)
    cen = centers.astype(np.float64)
    pw = powers.astype(np.int64)
    cc = (cen ** 2).sum(-1)
    absc = np.abs(co)
    lnc = np.where(absc > 0, np.log(np.where(absc > 0, absc, 1.0)), -745.0)
    sgn = np.sign(co)

    alf = al.reshape(-1)                       # r = a*P + p
    Wx = np.zeros((3, A * P))
    Wsq = np.zeros((3, A * P))
    for c in range(3):
        Wx[c] = 2.0 * alf * np.repeat(cen[:, c], P)
        Wsq[c] = -alf
    W1 = (-alf * np.repeat(cc, P) + lnc.reshape(-1))[None, :]

    def wstack(Wx, Wsq, W1):
        """20-row bf16 hi/lo weight stack matching the rr basis rows
        [xh, xl, sqh, sql, sqh, xh, 1, 1]."""
        WxH, WxL = _split_hl(Wx)
        WsqH, WsqL = _split_hl(Wsq)
        W1H, W1L = _split_hl(W1)
        return _bf(np.concatenate(
            [WxH, WxH, WsqH, WsqH, WsqL, WxL, W1H, W1L]))

    w = wstack(Wx, Wsq, W1)
    sc = 128.0 * LOG2E
    ws = wstack(Wx * sc, Wsq * sc,
                W1 * sc + 128.0 * (127.0 - SDELTA) + 0.5)

    # sign scatter S [128, NRT*128]
    smat = np.zeros((NRT, 128, 128))
    r = np.arange(A * P)
    t_of_r = r // 128
    m_of_r = (r // P) - np.where(t_of_r < NRT // 2, 0, 128)
    smat[t_of_r, r % 128, m_of_r] = sgn.reshape(-1)
    s2 = _bf(np.ascontiguousarray(
        smat.transpose(1, 0, 2).reshape(128, NRT * 128)))

    # angular monomial coefficients -> 81-row bf16 hi/lo [81, A]
    polc = np.zeros((A, 3, 3))
    for c in range(3):
        l = pw[:, c]
        polc[l == 0, c, 0] = 1.0
        polc[l == 1, c, 0] = -cen[l == 1, c]
        polc[l == 1, c, 1] = 1.0
        polc[l == 2, c, 0] = cen[l == 2, c] ** 2
        polc[l == 2, c, 1] = -2 * cen[l == 2, c]
        polc[l == 2, c, 2] = 1.0
    q27 = np.zeros((27, A))
    m = 0
    for ex in range(3):
        for ey in range(3):
            for ez in range(3):
                q27[m] = polc[:, 0, ex] * polc[:, 1, ey] * polc[:, 2, ez]
                m += 1
    qh, ql = _split_hl(q27)
    q81 = _bf(np.concatenate([qh, qh, ql]))
    return w, ws, s2, q81


def _basis(pos_shard):
    """Per-core basis: rr [20, I] bf16 and m81 [81, I] bf16."""
    X = pos_shard.reshape(I, 3).T.astype(np.float64)    # [3, I]
    sq = X * X
    xh, xl = _split_hl(X)
    sqh, sql = _split_hl(sq)
    one = np.ones((1, I))
    rr = _bf(np.concatenate([xh, xl, sqh, sql, sqh, xh, one, one]))
    mon = np.empty((27, I))
    m = 0
    for ex in range(3):
        for ey in range(3):
            for ez in range(3):
                mon[m] = (X[0] ** ex) * (X[1] ** ey) * (X[2] ** ez)
                m += 1
    mh, ml = _split_hl(mon)
    m81 = _bf(np.concatenate([mh, ml, mh]))
    return rr, m81


LAST_RESULT = None


def kernel(pos, centers, exps, coeffs, powers):
    global LAST_RESULT
    import os
    try:
        from antenv.axon_hooks import get_axon_ntff_profile_hook  # noqa: F401
    except ImportError:
        os.environ["BASS_NEVER_TRACE"] = "1"
    from concourse.bass_utils import run_bass_kernel_spmd

    pos = np.asarray(pos, dtype=np.float32)
    centers = np.asarray(centers, dtype=np.float32)
    exps = np.asarray(exps, dtype=np.float32)
    coeffs = np.asarray(coeffs, dtype=np.float32)
    powers = np.asarray(powers)

    if "nc" not in _CACHE:
        _CACHE["nc"] = _build_nc()
    nc = _CACHE["nc"]

    w, ws, s2, q81 = _consts(centers, exps, coeffs, powers)
    in_maps = []
    for ci in range(NCORES):
        rr, m81 = _basis(pos[ci * BS:(ci + 1) * BS])
        zin = np.concatenate([w, rr, ws], axis=1)
        ain = np.concatenate([q81, m81], axis=1)
        in_maps.append({"zin": np.ascontiguousarray(zin),
                        "ain": np.ascontiguousarray(ain),
                        "s": s2})

    res = run_bass_kernel_spmd(nc, in_maps, core_ids=list(range(NCORES)))
    LAST_RESULT = res
    out = np.stack([np.asarray(res.results[ci]["out"]).astype(np.float32)
                    for ci in range(NCORES)], axis=0)     # [8, 128, 2, I]
    # [8, 128(r), 2(h), BS*NEL] with a = h*128+r -> [B, NEL, A]
    out = out.transpose(0, 2, 1, 3).reshape(NCORES, A, BS, NEL)
    out = out.transpose(0, 2, 3, 1)
    return np.ascontiguousarray(out).reshape(B, NEL, A)


# revision 13
# speedup vs baseline: 1.2715x; 1.2715x over previous
"""AO layer kernel for Trainium2 (8 NeuronCores, data-parallel over walkers).

Math: out[b,n,a] = ang(a, r) * rad(a, r),  r = pos[b,n] - centers[a]
  rad = sum_p coeffs[a,p] * exp(-exps[a,p] * |r|^2)
  ang = prod_c r_c^powers[a,c],  powers in {0,1,2}

Design (per core, i = flattened (b,n) index, I=2048, 4 i-tiles of 512):
  z[(a,p), i] = W^T RR    (PE bf16 hi/lo 20-row basis, 12 r-tiles/i-tile)
  E = |c|exp(z)           split between ACT (exact exp, bf16 out) and DVE
                          (Schraudolph: weights pre-scaled so z lands in
                          bf16-bit space; clamp+int16-truncate via
                          tensor_scalar_max, bits reinterpreted as bf16)
  rad[a, i]   = S^T E     (PE bf16 accumulate, coeff signs inside S)
  ang[a, i]   = Q^T M     (PE single bf16 hi/lo matmul over the 27-monomial
                          tensor-product basis x^i y^j z^k, 81 rows)
  out = ang*rad           (DVE tensor_mul, ang copied PSUM->SBUF first since
                          TensorTensor may read only one PSUM input)
  Exp ops span 3 r-tiles ([128,1536]) to amortize per-op overhead.
  PSUM: 2x zp[128,1536] (6 banks) + rad[128,512] + ang[128,512].
  Output DRAM layout [128, 2, I] so each i-tile ships as one DMA.
"""

import numpy as np
import ml_dtypes

B, NEL, A, P = 512, 32, 256, 6
NCORES = 8
BS = B // NCORES          # 64 walkers per core
I = BS * NEL              # 2048 i per core
ITILE = 512
NIT = I // ITILE          # 4
NRT = 12                  # r-tiles of 128 (a,p) rows
KZ = 20                   # z basis rows (bf16 hi/lo)
KM = 81                   # monomial basis rows (bf16 hi/lo x3)
LOG2E = 1.4426950408889634
SDELTA = 0.058            # Schraudolph centering

# exp-group -> engine: (it, h, g) in set => DVE Schraudolph, else ACT exp.
DVE_UNITS = {(0, 1, 1), (1, 0, 1), (2, 1, 0), (3, 0, 0)}
# ang PSUM->SBUF copy engine per (it, h): in set => ACT Copy, else DVE copy
ACT_COPY_UNITS = {(0, 0), (2, 1)}

# packed input layouts
ZIN_WS = 0                # [0:1536)       Schraudolph weights (bf16 hi/lo)
ZIN_RR = A * P            # [1536:3584)    rr basis (bf16 hi/lo)
ZIN_COLS = I + A * P
K8 = 60                   # fp8 DoubleRow contraction rows per plane
AIN_Q = 0                 # [0:256)        q81
AIN_M = A                 # [256:2304)     m81
AIN_COLS = A + I

_CACHE = {}


def _bf(v):
    return np.asarray(v, np.float64).astype(ml_dtypes.bfloat16)


def _split_hl(w):
    wh = _bf(w).astype(np.float64)
    wl = _bf(np.asarray(w, np.float64) - wh).astype(np.float64)
    return wh, wl


_F8 = ml_dtypes.float8_e4m3


def _q8(v):
    return np.clip(v, -240, 240).astype(_F8).astype(np.float64)


def _pieces(v, n):
    """Successive fp8 residual pieces (true units), per-element mantissas."""
    v = np.asarray(v, np.float64)
    m = np.abs(v).max()
    g = int(np.floor(np.log2(224.0 / m))) if m > 0 else 0
    r = v.copy()
    out = []
    for _ in range(n):
        q = _q8(r * 2.0 ** g) * 2.0 ** -g
        out.append(q)
        r = r - q
        m = np.abs(r).max()
        if m > 1e-300:
            g = int(np.floor(np.log2(224.0 / m)))
    return out


def _fp8_rows(bfuncs, Wall):
    """fp8 DoubleRow z decomposition. bfuncs: [7, I] basis values
    [x,y,z,x2,y2,z2,1]; Wall: [7, C] product weights. Returns
    (RB [K8*2, I], RW [K8*2, C]) fp8 arrays, padded."""
    rows_b, rows_w = [], []
    for k in range(7):
        b, w = bfuncs[k], Wall[k]
        bp = _pieces(b, 5)
        wp = _pieces(w, 5)
        exact_b = k == 6
        for i in range(1, 6):
            for j in range(1, (1 if exact_b else 5) + 1):
                if i + j > (6 if exact_b else 7):
                    continue
                bt, wt = bp[j - 1], wp[i - 1]
                bm, wm = np.abs(bt).max(), np.abs(wt).max()
                if bm == 0 or wm == 0:
                    continue
                db = int(round(0.5 * np.log2(wm / bm)))
                db = min(db, int(np.floor(np.log2(240.0 / bm))))
                db = max(db, -int(np.floor(np.log2(240.0 / wm))))
                rows_b.append(_q8(bt * 2.0 ** db))
                rows_w.append(_q8(wt * 2.0 ** -db))
    R = len(rows_b)
    assert R <= 2 * K8, R
    RB = np.zeros((2 * K8, len(rows_b[0])), np.float64)
    RW = np.zeros((2 * K8, len(rows_w[0])), np.float64)
    RB[:R] = np.array(rows_b)
    RW[:R] = np.array(rows_w)
    # row r -> (k = r % K8, plane = r // K8)
    RB = RB.reshape(2, K8, -1).transpose(1, 0, 2)
    RW = RW.reshape(2, K8, -1).transpose(1, 0, 2)
    return RB.astype(_F8), RW.astype(_F8)


def _build_nc():
    import concourse.bass as bass
    import concourse.bacc as bacc
    import concourse.tile as tile
    import concourse.mybir as mybir

    f32 = mybir.dt.float32
    bf16 = mybir.dt.bfloat16
    i16 = mybir.dt.int16
    EXP = mybir.ActivationFunctionType.Exp
    CPY = mybir.ActivationFunctionType.Copy
    PSUM = bass.MemorySpace.PSUM

    nc = bacc.Bacc("TRN2", target_bir_lowering=False, debug=False,
                   num_devices=NCORES)

    fp8 = mybir.dt.float8e4
    DR = mybir.MatmulPerfMode.DoubleRow
    zin_d = nc.declare_dram_parameter("zin", [KZ, ZIN_COLS], bf16,
                                      isOutput=False)
    w8_d = nc.declare_dram_parameter("w8", [K8, 2, A * P], fp8,
                                     isOutput=False)
    r8_d = nc.declare_dram_parameter("r8", [K8, 2, I], fp8, isOutput=False)
    ain_d = nc.declare_dram_parameter("ain", [KM, AIN_COLS], bf16,
                                      isOutput=False)
    s_d = nc.declare_dram_parameter("s", [128, NRT * 128], bf16,
                                    isOutput=False)
    out_d = nc.declare_dram_parameter("out", [128, 2, I], bf16, isOutput=True)

    with tile.TileContext(nc) as tc:
        with (
            tc.tile_pool(name="const", bufs=1) as const,
            tc.tile_pool(name="zp", bufs=2, space=PSUM) as zp,
            tc.tile_pool(name="radp", bufs=1, space=PSUM) as radp,
            tc.tile_pool(name="angp", bufs=1, space=PSUM) as angp,
            tc.tile_pool(name="ep", bufs=4) as ep,
            tc.tile_pool(name="ap", bufs=3) as ap,
            tc.tile_pool(name="op", bufs=3) as op,
        ):
            # PE warmup: dummy matmuls keep the tensor engine's ramp state
            # hot while the input DMAs land, so real matmuls cost at 2.4GHz
            wu_sb = const.tile([1, 128], bf16, tag="wu")
            nc.vector.memset(wu_sb[:], 0.0)
            zin_sb = const.tile([KZ, ZIN_COLS], bf16, tag="zin")
            w8_sb = const.tile([K8, 2, A * P], fp8, tag="w8")
            r8_sb = const.tile([K8, 2, I], fp8, tag="r8")
            ain_sb = const.tile([KM, AIN_COLS], bf16, tag="ain")
            s_sb = const.tile([128, NRT * 128], bf16, tag="s")
            # ordered for just-in-time arrival (HWDGE is serialized)
            nc.sync.dma_start(w8_sb[:], w8_d[:, :, :])
            nc.scalar.dma_start(r8_sb[:, :, 0:2 * ITILE],
                                r8_d[:, :, 0:2 * ITILE])
            nc.sync.dma_start(s_sb[:], s_d[:, :])
            nc.scalar.dma_start(ain_sb[:, 0:AIN_M + ITILE],
                                ain_d[:, 0:AIN_M + ITILE])
            nc.sync.dma_start(zin_sb[:], zin_d[:, :])
            nc.scalar.dma_start(r8_sb[:, :, 2 * ITILE:],
                                r8_d[:, :, 2 * ITILE:])
            nc.sync.dma_start(ain_sb[:, AIN_M + ITILE:],
                                ain_d[:, AIN_M + ITILE:])

            wu_ps = zp.tile([128, 3 * ITILE], f32, tag="z")
            for j in range(40):
                c0 = (j % 3) * ITILE
                nc.tensor.matmul(wu_ps[0:1, c0:c0 + 128], wu_sb[:, 0:1],
                                 wu_sb[:], start=True, stop=True)

            for it in range(NIT):
                i0 = it * ITILE
                ri = zin_sb[:, ZIN_RR + i0:ZIN_RR + i0 + ITILE]
                for h in range(2):
                    last = it == NIT - 1 and h == 1
                    rad_t = radp.tile([128, ITILE], f32, tag="rad")
                    ang_t = angp.tile([128, ITILE], f32, tag="ang")
                    es = []
                    for g in range(2):
                        dve = (it, h, g) in DVE_UNITS
                        z_t = zp.tile([128, 3 * ITILE], f32, tag="z")
                        for j in range(3):
                            rt = h * 6 + g * 3 + j
                            if dve:
                                w0 = ZIN_WS + rt * 128
                                nc.tensor.matmul(
                                    z_t[:, j * ITILE:(j + 1) * ITILE],
                                    zin_sb[:, w0:w0 + 128], ri,
                                    start=True, stop=True)
                            else:
                                nc.tensor.matmul(
                                    z_t[:, j * ITILE:(j + 1) * ITILE],
                                    w8_sb[:, :, rt * 128:(rt + 1) * 128],
                                    r8_sb[:, :, i0:i0 + ITILE],
                                    start=True, stop=True, perf_mode=DR)
                        if dve:
                            e_t = ep.tile([128, 3 * ITILE], i16, tag="e")
                            nc.vector.tensor_scalar_max(e_t[:], z_t[:], 0.0)
                            es.append(e_t[:].bitcast(bf16))
                        else:
                            e_t = ep.tile([128, 3 * ITILE], bf16, tag="e")
                            nc.scalar.activation(e_t[:], z_t[:], EXP)
                            es.append(e_t[:])
                    # angular: one bf16 hi/lo matmul
                    if True:
                        nc.tensor.matmul(
                            ang_t[:],
                            ain_sb[:, AIN_Q + h * 128:AIN_Q + (h + 1) * 128],
                            ain_sb[:, AIN_M + i0:AIN_M + i0 + ITILE],
                            start=True, stop=True)
                    ang_sb = ap.tile([128, ITILE], bf16, tag="angsb")
                    if (it, h) in ACT_COPY_UNITS:
                        nc.scalar.activation(ang_sb[:], ang_t[:], CPY)
                    else:
                        nc.vector.tensor_copy(ang_sb[:], ang_t[:])
                    # radial: 6 accumulating sign-matmul passes
                    for g in range(2):
                        for j in range(3):
                            rt = h * 6 + g * 3 + j
                            nc.tensor.matmul(
                                rad_t[:],
                                s_sb[:, rt * 128:(rt + 1) * 128],
                                es[g][:, j * ITILE:(j + 1) * ITILE],
                                start=(g == 0 and j == 0),
                                stop=(g == 1 and j == 2))
                    if h == 0:
                        o_t = op.tile([128, 2, ITILE], bf16, tag="o")
                    if last:
                        # tail: ship h0 + most of h1 early; tiny final piece
                        # minimizes the post-compute DMA latency
                        H = 384
                        nc.vector.tensor_mul(
                            o_t[:, 1, 0:H], ang_sb[:, 0:H], rad_t[:, 0:H])
                        nc.scalar.dma_start(
                            out_d[:, 0:1, i0:i0 + ITILE], o_t[:, 0:1, :])
                        nc.scalar.dma_start(
                            out_d[:, 1, i0:i0 + H], o_t[:, 1, 0:H])
                        nc.vector.tensor_mul(
                            o_t[:, 1, H:], ang_sb[:, H:], rad_t[:, H:])
                        nc.sync.dma_start(
                            out_d[:, 1, i0 + H:i0 + ITILE], o_t[:, 1, H:])
                    else:
                        nc.vector.tensor_mul(o_t[:, h, :], ang_sb[:],
                                             rad_t[:])
                        if h == 1:
                            nc.sync.dma_start(
                                out_d[:, :, i0:i0 + ITILE], o_t[:])

    nc.compile()
    return nc


def _consts(centers, exps, coeffs, powers):
    """Device weight tensors (shared across cores)."""
    al = exps.astype(np.float64)
    co = coeffs.astype(np.float64)
    cen = centers.astype(np.float64)
    pw = powers.astype(np.int64)
    cc = (cen ** 2).sum(-1)
    absc = np.abs(co)
    lnc = np.where(absc > 0, np.log(np.where(absc > 0, absc, 1.0)), -745.0)
    sgn = np.sign(co)

    alf = al.reshape(-1)                       # r = a*P + p
    Wx = np.zeros((3, A * P))
    Wsq = np.zeros((3, A * P))
    for c in range(3):
        Wx[c] = 2.0 * alf * np.repeat(cen[:, c], P)
        Wsq[c] = -alf
    W1 = (-alf * np.repeat(cc, P) + lnc.reshape(-1))[None, :]

    def wstack(Wx, Wsq, W1):
        """20-row bf16 hi/lo weight stack matching the rr basis rows
        [xh, xl, sqh, sql, sqh, xh, 1, 1]."""
        WxH, WxL = _split_hl(Wx)
        WsqH, WsqL = _split_hl(Wsq)
        W1H, W1L = _split_hl(W1)
        return _bf(np.concatenate(
            [WxH, WxH, WsqH, WsqH, WsqL, WxL, W1H, W1L]))

    sc = 128.0 * LOG2E
    ws = wstack(Wx * sc, Wsq * sc,
                W1 * sc + 128.0 * (127.0 - SDELTA) + 0.5)
    W7 = np.concatenate([Wx, Wsq, W1], axis=0)   # [7, A*P] product weights

    # sign scatter S [128, NRT*128]
    smat = np.zeros((NRT, 128, 128))
    r = np.arange(A * P)
    t_of_r = r // 128
    m_of_r = (r // P) - np.where(t_of_r < NRT // 2, 0, 128)
    smat[t_of_r, r % 128, m_of_r] = sgn.reshape(-1)
    s2 = _bf(np.ascontiguousarray(
        smat.transpose(1, 0, 2).reshape(128, NRT * 128)))

    # angular monomial coefficients -> 81-row bf16 hi/lo [81, A]
    polc = np.zeros((A, 3, 3))
    for c in range(3):
        l = pw[:, c]
        polc[l == 0, c, 0] = 1.0
        polc[l == 1, c, 0] = -cen[l == 1, c]
        polc[l == 1, c, 1] = 1.0
        polc[l == 2, c, 0] = cen[l == 2, c] ** 2
        polc[l == 2, c, 1] = -2 * cen[l == 2, c]
        polc[l == 2, c, 2] = 1.0
    q27 = np.zeros((27, A))
    m = 0
    for ex in range(3):
        for ey in range(3):
            for ez in range(3):
                q27[m] = polc[:, 0, ex] * polc[:, 1, ey] * polc[:, 2, ez]
                m += 1
    qh, ql = _split_hl(q27)
    q81 = _bf(np.concatenate([qh, qh, ql]))
    return W7, ws, s2, q81


def _basis(pos_shard):
    """Per-core basis: rr [20, I] bf16 and m81 [81, I] bf16."""
    X = pos_shard.reshape(I, 3).T.astype(np.float64)    # [3, I]
    sq = X * X
    xh, xl = _split_hl(X)
    sqh, sql = _split_hl(sq)
    one = np.ones((1, I))
    rr = _bf(np.concatenate([xh, xl, sqh, sql, sqh, xh, one, one]))
    mon = np.empty((27, I))
    m = 0
    for ex in range(3):
        for ey in range(3):
            for ez in range(3):
                mon[m] = (X[0] ** ex) * (X[1] ** ey) * (X[2] ** ez)
                m += 1
    mh, ml = _split_hl(mon)
    m81 = _bf(np.concatenate([mh, ml, mh]))
    b7 = np.concatenate([X, sq, one])            # [7, I]
    return rr, m81, b7


LAST_RESULT = None


def kernel(pos, centers, exps, coeffs, powers):
    global LAST_RESULT
    import os
    try:
        from antenv.axon_hooks import get_axon_ntff_profile_hook  # noqa: F401
    except ImportError:
        os.environ["BASS_NEVER_TRACE"] = "1"
    from concourse.bass_utils import run_bass_kernel_spmd

    pos = np.asarray(pos, dtype=np.float32)
    centers = np.asarray(centers, dtype=np.float32)
    exps = np.asarray(exps, dtype=np.float32)
    coeffs = np.asarray(coeffs, dtype=np.float32)
    powers = np.asarray(powers)

    if "nc" not in _CACHE:
        _CACHE["nc"] = _build_nc()
    nc = _CACHE["nc"]

    W7, ws, s2, q81 = _consts(centers, exps, coeffs, powers)
    in_maps = []
    for ci in range(NCORES):
        rr, m81, b7 = _basis(pos[ci * BS:(ci + 1) * BS])
        r8, w8 = _fp8_rows(b7, W7)
        zin = np.concatenate([ws, rr], axis=1)
        ain = np.concatenate([q81, m81], axis=1)
        in_maps.append({"zin": np.ascontiguousarray(zin),
                        "ain": np.ascontiguousarray(ain),
                        "r8": np.ascontiguousarray(r8),
                        "w8": np.ascontiguousarray(w8),
                        "s": s2})

    res = run_bass_kernel_spmd(nc, in_maps, core_ids=list(range(NCORES)))
    LAST_RESULT = res
    out = np.stack([np.asarray(res.results[ci]["out"]).astype(np.float32)
                    for ci in range(NCORES)], axis=0)     # [8, 128, 2, I]
    # [8, 128(r), 2(h), BS*NEL] with a = h*128+r -> [B, NEL, A]
    out = out.transpose(0, 2, 1, 3).reshape(NCORES, A, BS, NEL)
    out = out.transpose(0, 2, 3, 1)
    return np.ascontiguousarray(out).reshape(B, NEL, A)


# revision 19
# speedup vs baseline: 1.3079x; 1.0286x over previous
"""AO layer kernel for Trainium2 (8 NeuronCores, data-parallel over walkers).

Math: out[b,n,a] = ang(a, r) * rad(a, r),  r = pos[b,n] - centers[a]
  rad = sum_p coeffs[a,p] * exp(-exps[a,p] * |r|^2)
  ang = prod_c r_c^powers[a,c],  powers in {0,1,2}

Design (per core, i = flattened (b,n) index, I=2048, 4 i-tiles of 512):
  z[(a,p), i] = W^T RR    (PE bf16 hi/lo 20-row basis, 12 r-tiles/i-tile)
  E = |c|exp(z)           split between ACT (exact exp, bf16 out) and DVE
                          (Schraudolph: weights pre-scaled so z lands in
                          bf16-bit space; clamp+int16-truncate via
                          tensor_scalar_max, bits reinterpreted as bf16)
  rad[a, i]   = S^T E     (PE bf16 accumulate, coeff signs inside S)
  ang[a, i]   = Q^T M     (PE single bf16 hi/lo matmul over the 27-monomial
                          tensor-product basis x^i y^j z^k, 81 rows)
  out = ang*rad           (DVE tensor_mul, ang copied PSUM->SBUF first since
                          TensorTensor may read only one PSUM input)
  Exp ops span 3 r-tiles ([128,1536]) to amortize per-op overhead.
  PSUM: 2x zp[128,1536] (6 banks) + rad[128,512] + ang[128,512].
  Output DRAM layout [128, 2, I] so each i-tile ships as one DMA.
"""

import numpy as np
import ml_dtypes

B, NEL, A, P = 512, 32, 256, 6
NCORES = 8
BS = B // NCORES          # 64 walkers per core
I = BS * NEL              # 2048 i per core
ITILE = 512
NIT = I // ITILE          # 4
NRT = 12                  # r-tiles of 128 (a,p) rows
KZ = 20                   # z basis rows (bf16 hi/lo)
KM = 81                   # monomial basis rows (bf16 hi/lo x3)
LOG2E = 1.4426950408889634
SDELTA = 0.058            # Schraudolph centering

# exp-group -> engine: (it, h, g) in set => DVE Schraudolph, else ACT exp.
DVE_UNITS = {(0, 1, 1), (1, 0, 1), (1, 1, 1), (2, 1, 0), (3, 0, 0)}
# ang PSUM->SBUF copy engine per (it, h): in set => ACT Copy, else DVE copy
ACT_COPY_UNITS = {(0, 0), (2, 1)}

# packed input layouts
ZIN_WS = 0                # [0:1536)       Schraudolph weights (bf16 hi/lo)
ZIN_RR = A * P            # [1536:3584)    rr basis (bf16 hi/lo)
ZIN_COLS = I + A * P
K8 = 60                   # fp8 DoubleRow contraction rows per plane
AIN_Q = 0                 # [0:256)        q81
AIN_M = A                 # [256:2304)     m81
AIN_COLS = A + I

_CACHE = {}


def _bf(v):
    return np.asarray(v, np.float64).astype(ml_dtypes.bfloat16)


def _split_hl(w):
    wh = _bf(w).astype(np.float64)
    wl = _bf(np.asarray(w, np.float64) - wh).astype(np.float64)
    return wh, wl


_F8 = ml_dtypes.float8_e4m3


def _q8(v):
    return np.clip(v, -240, 240).astype(_F8).astype(np.float64)


def _pieces(v, n):
    """Successive fp8 residual pieces (true units), per-element mantissas."""
    v = np.asarray(v, np.float64)
    m = np.abs(v).max()
    g = int(np.floor(np.log2(224.0 / m))) if m > 0 else 0
    r = v.copy()
    out = []
    for _ in range(n):
        q = _q8(r * 2.0 ** g) * 2.0 ** -g
        out.append(q)
        r = r - q
        m = np.abs(r).max()
        if m > 1e-300:
            g = int(np.floor(np.log2(224.0 / m)))
    return out


def _fp8_rows(bfuncs, Wall):
    """fp8 DoubleRow z decomposition. bfuncs: [7, I] basis values
    [x,y,z,x2,y2,z2,1]; Wall: [7, C] product weights. Returns
    (RB [K8*2, I], RW [K8*2, C]) fp8 arrays, padded."""
    rows_b, rows_w = [], []
    for k in range(7):
        b, w = bfuncs[k], Wall[k]
        bp = _pieces(b, 5)
        wp = _pieces(w, 5)
        exact_b = k == 6
        for i in range(1, 6):
            for j in range(1, (1 if exact_b else 5) + 1):
                if i + j > (6 if exact_b else 7):
                    continue
                bt, wt = bp[j - 1], wp[i - 1]
                bm, wm = np.abs(bt).max(), np.abs(wt).max()
                if bm == 0 or wm == 0:
                    continue
                db = int(round(0.5 * np.log2(wm / bm)))
                db = min(db, int(np.floor(np.log2(240.0 / bm))))
                db = max(db, -int(np.floor(np.log2(240.0 / wm))))
                rows_b.append(_q8(bt * 2.0 ** db))
                rows_w.append(_q8(wt * 2.0 ** -db))
    R = len(rows_b)
    assert R <= 2 * K8, R
    RB = np.zeros((2 * K8, len(rows_b[0])), np.float64)
    RW = np.zeros((2 * K8, len(rows_w[0])), np.float64)
    RB[:R] = np.array(rows_b)
    RW[:R] = np.array(rows_w)
    # row r -> (k = r % K8, plane = r // K8)
    RB = RB.reshape(2, K8, -1).transpose(1, 0, 2)
    RW = RW.reshape(2, K8, -1).transpose(1, 0, 2)
    return RB.astype(_F8), RW.astype(_F8)


def _build_nc():
    import concourse.bass as bass
    import concourse.bacc as bacc
    import concourse.tile as tile
    import concourse.mybir as mybir

    f32 = mybir.dt.float32
    bf16 = mybir.dt.bfloat16
    i16 = mybir.dt.int16
    EXP = mybir.ActivationFunctionType.Exp
    CPY = mybir.ActivationFunctionType.Copy
    PSUM = bass.MemorySpace.PSUM

    nc = bacc.Bacc("TRN2", target_bir_lowering=False, debug=False,
                   num_devices=NCORES)

    fp8 = mybir.dt.float8e4
    DR = mybir.MatmulPerfMode.DoubleRow
    zin_d = nc.declare_dram_parameter("zin", [KZ, ZIN_COLS], bf16,
                                      isOutput=False)
    w8_d = nc.declare_dram_parameter("w8", [K8, 2, A * P], fp8,
                                     isOutput=False)
    r8_d = nc.declare_dram_parameter("r8", [K8, 2, I], fp8, isOutput=False)
    ain_d = nc.declare_dram_parameter("ain", [KM, AIN_COLS], bf16,
                                      isOutput=False)
    s_d = nc.declare_dram_parameter("s", [128, NRT * 128], bf16,
                                    isOutput=False)
    out_d = nc.declare_dram_parameter("out", [128, 2, I], bf16, isOutput=True)

    with tile.TileContext(nc) as tc:
        with (
            tc.tile_pool(name="const", bufs=1) as const,
            tc.tile_pool(name="zp", bufs=2, space=PSUM) as zp,
            tc.tile_pool(name="rap", bufs=2, space=PSUM) as rap,
            tc.tile_pool(name="ep", bufs=4) as ep,
            tc.tile_pool(name="ap", bufs=3) as ap,
            tc.tile_pool(name="op", bufs=3) as op,
        ):
            # PE warmup: dummy matmuls keep the tensor engine's ramp state
            # hot while the input DMAs land, so real matmuls cost at 2.4GHz
            wu_sb = const.tile([1, 128], bf16, tag="wu")
            nc.vector.memset(wu_sb[:], 0.0)
            zin_sb = const.tile([KZ, ZIN_COLS], bf16, tag="zin")
            w8_sb = const.tile([K8, 2, A * P], fp8, tag="w8")
            r8_sb = const.tile([K8, 2, I], fp8, tag="r8")
            ain_sb = const.tile([KM, AIN_COLS], bf16, tag="ain")
            s_sb = const.tile([128, NRT * 128], bf16, tag="s")
            # ordered for just-in-time arrival (HWDGE is serialized)
            nc.sync.dma_start(w8_sb[:], w8_d[:, :, :])
            nc.scalar.dma_start(r8_sb[:, :, 0:2 * ITILE],
                                r8_d[:, :, 0:2 * ITILE])
            nc.sync.dma_start(s_sb[:, 0:6 * 128], s_d[:, 0:6 * 128])
            nc.scalar.dma_start(ain_sb[:, 0:AIN_M + ITILE],
                                ain_d[:, 0:AIN_M + ITILE])
            nc.sync.dma_start(zin_sb[:], zin_d[:, :])
            nc.sync.dma_start(s_sb[:, 6 * 128:], s_d[:, 6 * 128:])
            nc.scalar.dma_start(r8_sb[:, :, 2 * ITILE:],
                                r8_d[:, :, 2 * ITILE:])
            nc.sync.dma_start(ain_sb[:, AIN_M + ITILE:],
                                ain_d[:, AIN_M + ITILE:])

            wu_ps = zp.tile([128, 3 * ITILE], f32, tag="z")
            with tc.high_priority(offset=-2000):
                for j in range(28):
                    c0 = (j % 3) * ITILE
                    nc.tensor.matmul(wu_ps[0:1, c0:c0 + 128], wu_sb[:, 0:1],
                                     wu_sb[:], start=True, stop=True)

            for it in range(NIT):
                i0 = it * ITILE
                ri = zin_sb[:, ZIN_RR + i0:ZIN_RR + i0 + ITILE]
                for h in range(2):
                    last = it == NIT - 1 and h == 1
                    # ang and rad time-share one PSUM tile: ang matmul ->
                    # copy out -> rad accumulation -> final mul
                    ra_t = rap.tile([128, ITILE], f32, tag="ra")
                    es = []
                    for g in range(2):
                        dve = (it, h, g) in DVE_UNITS
                        z_t = zp.tile([128, 3 * ITILE], f32, tag="z")
                        for j in range(3):
                            rt = h * 6 + g * 3 + j
                            if dve:
                                w0 = ZIN_WS + rt * 128
                                nc.tensor.matmul(
                                    z_t[:, j * ITILE:(j + 1) * ITILE],
                                    zin_sb[:, w0:w0 + 128], ri,
                                    start=True, stop=True)
                            else:
                                nc.tensor.matmul(
                                    z_t[:, j * ITILE:(j + 1) * ITILE],
                                    w8_sb[:, :, rt * 128:(rt + 1) * 128],
                                    r8_sb[:, :, i0:i0 + ITILE],
                                    start=True, stop=True, perf_mode=DR)
                        if dve:
                            e_t = ep.tile([128, 3 * ITILE], i16, tag="e")
                            nc.vector.tensor_scalar_max(e_t[:], z_t[:], 0.0)
                            es.append(e_t[:].bitcast(bf16))
                        else:
                            e_t = ep.tile([128, 3 * ITILE], bf16, tag="e")
                            nc.scalar.activation(e_t[:], z_t[:], EXP)
                            es.append(e_t[:])
                    # angular: one bf16 hi/lo matmul
                    nc.tensor.matmul(
                        ra_t[:],
                        ain_sb[:, AIN_Q + h * 128:AIN_Q + (h + 1) * 128],
                        ain_sb[:, AIN_M + i0:AIN_M + i0 + ITILE],
                        start=True, stop=True)
                    ang_sb = ap.tile([128, ITILE], bf16, tag="angsb")
                    if (it, h) in ACT_COPY_UNITS:
                        nc.scalar.activation(ang_sb[:], ra_t[:], CPY)
                    else:
                        nc.vector.tensor_copy(ang_sb[:], ra_t[:])
                    # radial: 6 accumulating sign-matmul passes reusing ra_t
                    for g in range(2):
                        for j in range(3):
                            rt = h * 6 + g * 3 + j
                            nc.tensor.matmul(
                                ra_t[:],
                                s_sb[:, rt * 128:(rt + 1) * 128],
                                es[g][:, j * ITILE:(j + 1) * ITILE],
                                start=(g == 0 and j == 0),
                                stop=(g == 1 and j == 2))
                    if h == 0:
                        o_t = op.tile([128, 2, ITILE], bf16, tag="o")
                    if last:
                        # tail: ship h0 + most of h1 early; tiny final piece
                        # minimizes the post-compute DMA latency
                        H = 448
                        nc.vector.tensor_mul(
                            o_t[:, 1, 0:H], ang_sb[:, 0:H], ra_t[:, 0:H])
                        nc.scalar.dma_start(
                            out_d[:, 0:1, i0:i0 + ITILE], o_t[:, 0:1, :])
                        nc.scalar.dma_start(
                            out_d[:, 1, i0:i0 + H], o_t[:, 1, 0:H])
                        nc.vector.tensor_mul(
                            o_t[:, 1, H:], ang_sb[:, H:], ra_t[:, H:])
                        nc.sync.dma_start(
                            out_d[:, 1, i0 + H:i0 + ITILE], o_t[:, 1, H:])
                    else:
                        nc.vector.tensor_mul(o_t[:, h, :], ang_sb[:],
                                             ra_t[:])
                        if h == 1:
                            nc.sync.dma_start(
                                out_d[:, :, i0:i0 + ITILE], o_t[:])

    nc.compile()
    return nc


def _consts(centers, exps, coeffs, powers):
    """Device weight tensors (shared across cores)."""
    al = exps.astype(np.float64)
    co = coeffs.astype(np.float64)
    cen = centers.astype(np.float64)
    pw = powers.astype(np.int64)
    cc = (cen ** 2).sum(-1)
    absc = np.abs(co)
    lnc = np.where(absc > 0, np.log(np.where(absc > 0, absc, 1.0)), -745.0)
    sgn = np.sign(co)

    alf = al.reshape(-1)                       # r = a*P + p
    Wx = np.zeros((3, A * P))
    Wsq = np.zeros((3, A * P))
    for c in range(3):
        Wx[c] = 2.0 * alf * np.repeat(cen[:, c], P)
        Wsq[c] = -alf
    W1 = (-alf * np.repeat(cc, P) + lnc.reshape(-1))[None, :]

    def wstack(Wx, Wsq, W1):
        """20-row bf16 hi/lo weight stack matching the rr basis rows
        [xh, xl, sqh, sql, sqh, xh, 1, 1]."""
        WxH, WxL = _split_hl(Wx)
        WsqH, WsqL = _split_hl(Wsq)
        W1H, W1L = _split_hl(W1)
        return _bf(np.concatenate(
            [WxH, WxH, WsqH, WsqH, WsqL, WxL, W1H, W1L]))

    sc = 128.0 * LOG2E
    ws = wstack(Wx * sc, Wsq * sc,
                W1 * sc + 128.0 * (127.0 - SDELTA) + 0.5)
    W7 = np.concatenate([Wx, Wsq, W1], axis=0)   # [7, A*P] product weights

    # sign scatter S [128, NRT*128]
    smat = np.zeros((NRT, 128, 128))
    r = np.arange(A * P)
    t_of_r = r // 128
    m_of_r = (r // P) - np.where(t_of_r < NRT // 2, 0, 128)
    smat[t_of_r, r % 128, m_of_r] = sgn.reshape(-1)
    s2 = _bf(np.ascontiguousarray(
        smat.transpose(1, 0, 2).reshape(128, NRT * 128)))

    # angular monomial coefficients -> 81-row bf16 hi/lo [81, A]
    polc = np.zeros((A, 3, 3))
    for c in range(3):
        l = pw[:, c]
        polc[l == 0, c, 0] = 1.0
        polc[l == 1, c, 0] = -cen[l == 1, c]
        polc[l == 1, c, 1] = 1.0
        polc[l == 2, c, 0] = cen[l == 2, c] ** 2
        polc[l == 2, c, 1] = -2 * cen[l == 2, c]
        polc[l == 2, c, 2] = 1.0
    q27 = np.zeros((27, A))
    m = 0
    for ex in range(3):
        for ey in range(3):
            for ez in range(3):
                q27[m] = polc[:, 0, ex] * polc[:, 1, ey] * polc[:, 2, ez]
                m += 1
    qh, ql = _split_hl(q27)
    q81 = _bf(np.concatenate([qh, qh, ql]))
    return W7, ws, s2, q81


def _basis(pos_shard):
    """Per-core basis: rr [20, I] bf16 and m81 [81, I] bf16."""
    X = pos_shard.reshape(I, 3).T.astype(np.float64)    # [3, I]
    sq = X * X
    xh, xl = _split_hl(X)
    sqh, sql = _split_hl(sq)
    one = np.ones((1, I))
    rr = _bf(np.concatenate([xh, xl, sqh, sql, sqh, xh, one, one]))
    mon = np.empty((27, I))
    m = 0
    for ex in range(3):
        for ey in range(3):
            for ez in range(3):
                mon[m] = (X[0] ** ex) * (X[1] ** ey) * (X[2] ** ez)
                m += 1
    mh, ml = _split_hl(mon)
    m81 = _bf(np.concatenate([mh, ml, mh]))
    b7 = np.concatenate([X, sq, one])            # [7, I]
    return rr, m81, b7


LAST_RESULT = None


def kernel(pos, centers, exps, coeffs, powers):
    global LAST_RESULT
    import os
    try:
        from antenv.axon_hooks import get_axon_ntff_profile_hook  # noqa: F401
    except ImportError:
        os.environ["BASS_NEVER_TRACE"] = "1"
    from concourse.bass_utils import run_bass_kernel_spmd

    pos = np.asarray(pos, dtype=np.float32)
    centers = np.asarray(centers, dtype=np.float32)
    exps = np.asarray(exps, dtype=np.float32)
    coeffs = np.asarray(coeffs, dtype=np.float32)
    powers = np.asarray(powers)

    if "nc" not in _CACHE:
        _CACHE["nc"] = _build_nc()
    nc = _CACHE["nc"]

    W7, ws, s2, q81 = _consts(centers, exps, coeffs, powers)
    in_maps = []
    for ci in range(NCORES):
        rr, m81, b7 = _basis(pos[ci * BS:(ci + 1) * BS])
        r8, w8 = _fp8_rows(b7, W7)
        zin = np.concatenate([ws, rr], axis=1)
        ain = np.concatenate([q81, m81], axis=1)
        in_maps.append({"zin": np.ascontiguousarray(zin),
                        "ain": np.ascontiguousarray(ain),
                        "r8": np.ascontiguousarray(r8),
                        "w8": np.ascontiguousarray(w8),
                        "s": s2})

    res = run_bass_kernel_spmd(nc, in_maps, core_ids=list(range(NCORES)))
    LAST_RESULT = res
    out = np.stack([np.asarray(res.results[ci]["out"]).astype(np.float32)
                    for ci in range(NCORES)], axis=0)     # [8, 128, 2, I]
    # [8, 128(r), 2(h), BS*NEL] with a = h*128+r -> [B, NEL, A]
    out = out.transpose(0, 2, 1, 3).reshape(NCORES, A, BS, NEL)
    out = out.transpose(0, 2, 3, 1)
    return np.ascontiguousarray(out).reshape(B, NEL, A)
